# revision 2
# baseline (speedup 1.0000x reference)
"""Causal self-attention (B=2, T=2048, D=1024, H=16) on 8 TRN2 NeuronCores.

v3 (shipped): v2 structure + race fix and pipeline changes.
  - RACE FIX: exp reads exactly the psum ranges the diagonal-group score
    matmuls wrote (per-j activations). v2 exp'd the whole tile, reading
    stale psum regions still tracked as another pool tenant's tile —
    CoreSim flags it and it corrupted HW output once (rel err 2e5).
  - z matmuls lag the score matmuls by LAG=2 k-groups (not 1), so exp(g-2)
    is always complete when z(g-2) issues: the in-order PE queue no longer
    ping-pongs with ACT every group-step.
  - Fillers (projection + output-projection chunks) are emitted only in the
    steps where the attention pipe has slack: the first LAG steps (no z yet)
    and the drain steps (no scores left) of each (q-block, head-pair).
  - All per-rep input tiles are double-buffered (cst pool bufs=2) so the
    next rep's DMA + projections overlap the previous rep's attention.
  - Output projection reuses each stationary operand for 2 matmuls.

Sharding: data parallel on batch (2 groups of 4 cores) x tensor parallel on
heads (4 heads per core). Each core computes X[b] -> partial_out for its 4
heads; the host sums the 4 partials per batch. No device collectives.

Per-core math (bf16 matmuls, f32 psum):
  qT/kT  [e,t] = Wqkv_localT.T @ X[b].T     (q pre-scaled by 1/8 on host)
  v      [t,d] = X[b] @ Wv_localT
  scoresT[k,q] = kT.T @ qT                  (contraction hd=64)
  attT   [k,q] = exp(scoresT) * causal-mask (no max subtraction; |s| = O(1))
  zT     [d,q] = v_ext.T @ attT             (ones column -> row 64 = denom)
  out    [t,e] = (zT/denom).T @ Wout_localT (host sums 4 partials per batch)
"""

import sys

sys.path.insert(0, "/opt/trn_rl_repo")

from collections import deque

import numpy as np

import concourse.bacc as bacc
import concourse.mybir as mybir
import concourse.tile as tile

B, T, D, H = 2, 2048, 1024, 16
HD = D // H              # 64
NCORES = 8
NGROUP = 4               # cores per batch (tensor-parallel group)
HPC = H // NGROUP        # heads per core = 4
DLOC = HPC * HD          # local model dims per core = 256
QB = 512                 # q-block (matmul moving free dim)
NQB = T // QB            # 4
KT = 128                 # k-tile (psum partitions)
NKT = T // KT            # 16
NDT = D // 128           # 8 d-tiles
LAG = 2                  # z trails scores by LAG k-groups

F32 = mybir.dt.float32
BF16 = mybir.dt.bfloat16


def build_program(reps=1):
    nc = bacc.Bacc("TRN2", target_bir_lowering=False, debug=False,
                   num_devices=NCORES)

    xt_d = nc.declare_dram_parameter("xt", [D, T], BF16, isOutput=False)
    wqkv_d = nc.declare_dram_parameter("wqkv", [D, 3 * DLOC], BF16, isOutput=False)
    wout_d = nc.declare_dram_parameter("wout", [DLOC, D], BF16, isOutput=False)
    mask_d = nc.declare_dram_parameter("masks", [KT, 128], BF16, isOutput=False)
    ones_d = nc.declare_dram_parameter("ones", [128, NKT, HPC, 1], BF16, isOutput=False)
    out_d = nc.declare_dram_parameter("out", [T, D], BF16, isOutput=True)

    with tile.TileContext(nc) as tc:
        with (
            tc.tile_pool(name="cst", bufs=2) as cst,
            tc.tile_pool(name="att", bufs=8) as attp,
            tc.tile_pool(name="sm", bufs=4) as smp,
            tc.tile_pool(name="ops", bufs=4) as opsp,
            tc.tile_pool(name="ps", bufs=3, space="PSUM") as ps,    # [128,2,512]
            tc.tile_pool(name="zps", bufs=2, space="PSUM") as zps,  # [128,512]
        ):
            def body(_i):
                xt = cst.tile([128, NDT, T], BF16, tag="xt")
                wqkv = cst.tile([128, NDT, 3 * DLOC], BF16, tag="wqkv")
                wout = cst.tile([128, DLOC // 128, D], BF16, tag="wout")
                masks = cst.tile([128, 128], BF16, tag="masks")
                qT = cst.tile([128, 2, T], BF16, tag="qT")
                kT = cst.tile([128, 2, T], BF16, tag="kT")
                vext = cst.tile([128, NKT, HPC, HD + 1], BF16, tag="vext")

                def load_inputs():
                    # split big loads across DMA queues; arrival order matches
                    # the order the projections consume the data
                    wqkv_r = wqkv_d.rearrange("(a p) e -> p a e", p=128)
                    xt_r = xt_d.rearrange("(a p) t -> p a t", p=128)
                    for dt_ in range(0, NDT, 2):
                        nc.sync.dma_start(wqkv[:, dt_:dt_ + 2, 0:2 * DLOC],
                                          wqkv_r[:, dt_:dt_ + 2, 0:2 * DLOC])
                        nc.sync.dma_start(xt[:, dt_:dt_ + 2, 0:QB],
                                          xt_r[:, dt_:dt_ + 2, 0:QB])
                    for dt_ in range(0, NDT, 2):   # v weight columns
                        nc.sync.dma_start(wqkv[:, dt_:dt_ + 2, 2 * DLOC:3 * DLOC],
                                          wqkv_r[:, dt_:dt_ + 2, 2 * DLOC:3 * DLOC])
                    for tchunk in range(1, 4):
                        sl = slice(tchunk * QB, (tchunk + 1) * QB)
                        nc.sync.dma_start(xt[:, :, sl], xt_r[:, :, sl])
                    nc.sync.dma_start(wout[:], wout_d.rearrange("(a p) e -> p a e", p=128))
                    nc.sync.dma_start(masks[:], mask_d[:])
                    nc.sync.dma_start(vext[:, :, :, HD:HD + 1], ones_d[:])

                def qk_chunks(tb):
                    """4 filler chunks: q j0, q j1(+copy), k j0, k j1(+copy)."""
                    state = {}

                    def mk(pair, j):
                        def f():
                            if j == 0:
                                state[pair] = ps.tile([128, 2, QB], F32, tag="ps",
                                                      name=f"pt_qk{tb}_{pair}")
                            pt = state[pair]
                            ecol = (pair * 2 + j) * 128
                            for dt_ in range(NDT):
                                nc.tensor.matmul(
                                    pt[:, j, :],
                                    wqkv[:, dt_, ecol:ecol + 128],
                                    xt[:, dt_, tb * QB:(tb + 1) * QB],
                                    start=(dt_ == 0), stop=(dt_ == NDT - 1),
                                )
                            if j == 1:
                                dst = qT if pair == 0 else kT
                                nc.vector.tensor_copy(
                                    dst[:, :, tb * QB:(tb + 1) * QB],
                                    state.pop(pair)[:])
                        return f

                    return [mk(p_, j_) for p_ in (0, 1) for j_ in (0, 1)]

                def v_chunks(tp):
                    """2 filler chunks (one per 128-t-chunk), each with copy."""
                    state = {}

                    def mk(j):
                        def f():
                            if j == 0:
                                state["pt"] = ps.tile([128, 2, QB], F32, tag="ps",
                                                      name=f"pt_v{tp}")
                            pt = state["pt"]
                            tch = tp * 2 + j
                            for dt_ in range(NDT):
                                nc.tensor.matmul(
                                    pt[:, j, 0:DLOC],
                                    xt[:, dt_, tch * 128:(tch + 1) * 128],
                                    wqkv[:, dt_, 2 * DLOC:3 * DLOC],
                                    start=(dt_ == 0), stop=(dt_ == NDT - 1),
                                )
                            nc.vector.tensor_copy(
                                vext[:, tch, :, 0:HD],
                                pt[:, j, 0:DLOC].rearrange("p (h d) -> p h d", h=HPC),
                            )
                        return f

                    return [mk(0), mk(1)]

                def proj_chunks(b):
                    return (qk_chunks(b) + v_chunks(2 * b) + v_chunks(2 * b + 1))

                load_inputs()
                zTn_prev = {}

                def oproj_chunks(qo):
                    """4 filler chunks, one per output t-chunk of 128 rows."""
                    zo = zTn_prev.pop(qo)

                    def mk(tch):
                        def f():
                            po = ps.tile([128, 2, QB], F32, tag="ps", name=f"po{qo}_{tch}")
                            # dt_ outer so each stationary zo slice feeds 2 MMs
                            for dt_ in range(2):
                                for et in range(2):
                                    nc.tensor.matmul(
                                        po[:, et, :],
                                        zo[:, dt_, tch * 128:(tch + 1) * 128],
                                        wout[:, dt_, et * QB:(et + 1) * QB],
                                        start=(dt_ == 0), stop=(dt_ == 1),
                                    )
                            ot = opsp.tile([128, 2, QB], BF16, tag="ot", name=f"ot{qo}_{tch}")
                            # GPSIMD cannot access PSUM: staging stays on DVE
                            nc.vector.tensor_copy(ot[:], po[:])
                            row = qo * QB + tch * 128
                            nc.sync.dma_start(
                                out_d[row:row + 128, :],
                                ot.rearrange("p a q -> p (a q)"))
                        return f

                    return [mk(t_) for t_ in range(QB // 128)]

                fillers = deque()

                def emit_filler(n=1):
                    for _ in range(n):
                        if fillers:
                            fillers.popleft()()

                # projections for blocks 0 and 1 up front; later blocks and
                # delayed output projections become attention fillers
                with nc.named_scope("proj_q0"):
                    for c in proj_chunks(0):
                        c()
                with nc.named_scope("proj_q1"):
                    for c in proj_chunks(1):
                        c()

                for qi in range(NQB):
                    if qi + 2 < NQB:
                        fillers.extend(proj_chunks(qi + 2))
                    if qi >= 1:
                        fillers.extend(oproj_chunks(qi - 1))

                    zTn = smp.tile([128, 2, QB], BF16, tag="zTn")
                    zTn_prev[qi] = zTn
                    G = 2 * (qi + 1)          # k-groups of 2 k-tiles
                    for p in range(2):        # head pairs (0,1) then (2,3)
                        zts = {}
                        att_tiles = {}
                        with nc.named_scope(f"att_q{qi}_p{p}"):
                            for g in range(G + LAG):
                                # diagonal k-tiles (last 2 groups): only
                                # cols >= the strip offset r are causal
                                def _r(gg, j):
                                    if gg < G - 2:
                                        return None
                                    return (gg - (G - 2)) * 256 + j * 128

                                def _lo(r):
                                    return 0 if r is None else r

                                for h in (2 * p, 2 * p + 1):
                                    off, hv = (h % 2) * 64, h // 2
                                    if g < G:
                                        if g == 0:
                                            zts[h] = zps.tile([128, QB], F32,
                                                              tag="zt", name=f"zt{h}")
                                        sc = ps.tile([128, 2, QB], F32, tag="ps")
                                        for j in range(2):
                                            kt_i = g * 2 + j
                                            lo = _lo(_r(g, j))
                                            nc.tensor.matmul(
                                                sc[:, j, lo:],
                                                kT[off:off + 64, hv,
                                                   kt_i * 128:(kt_i + 1) * 128],
                                                qT[off:off + 64, hv,
                                                   qi * QB + lo:(qi + 1) * QB],
                                                start=True, stop=True,
                                            )
                                        at = attp.tile([128, 2, QB], BF16, tag="at")
                                        if _r(g, 0) is None:
                                            # non-diagonal: sc fully written
                                            nc.scalar.activation(
                                                at[:], sc[:],
                                                mybir.ActivationFunctionType.Exp)
                                        else:
                                            # diagonal: exp exactly the ranges
                                            # the score matmuls wrote — the
                                            # rest of sc is stale psum (race)
                                            for j in range(2):
                                                lo = _r(g, j)
                                                nc.scalar.activation(
                                                    at[:, j, lo:], sc[:, j, lo:],
                                                    mybir.ActivationFunctionType.Exp)
                                        for j in range(2):
                                            r = _r(g, j)
                                            if r is None:
                                                continue
                                            nc.gpsimd.tensor_mul(
                                                at[:, j, r:r + 128],
                                                at[:, j, r:r + 128],
                                                masks[:, 0:128])
                                        att_tiles[h, g] = at
                                # pipe has PE slack only while z hasn't
                                # started (g < LAG) or scores are done
                                if g < LAG or g >= G:
                                    emit_filler(1)
                                for h in (2 * p, 2 * p + 1):
                                    off, hv = (h % 2) * 64, h // 2
                                    gz = g - LAG
                                    if gz >= 0:
                                        ap = att_tiles.pop((h, gz))
                                        for j in range(2):
                                            kt_i = gz * 2 + j
                                            lo = _lo(_r(gz, j))
                                            nc.tensor.matmul(
                                                zts[h][0:HD + 1, lo:],
                                                vext[:, kt_i, h, :],
                                                ap[:, j, lo:],
                                                start=(gz == 0 and j == 0),
                                                stop=(gz == G - 1 and j == 1),
                                            )
                                    if g == G + LAG - 1:
                                        zt = zts[h]
                                        scr = smp.tile([128, QB], F32, tag="scr")
                                        bc = smp.tile([128, QB], F32, tag="bc")
                                        nc.vector.reciprocal(scr[0:1, :], zt[HD:HD + 1, :])
                                        nc.gpsimd.partition_broadcast(
                                            bc[:], scr[0:1, :], channels=128)
                                        nc.vector.tensor_mul(
                                            zTn[off:off + 64, hv, :],
                                            zt[0:HD, :], bc[off:off + 64, :])

                # drain leftovers, then the last block's output projection
                emit_filler(len(fillers))
                with nc.named_scope("oproj_tail"):
                    for c in oproj_chunks(NQB - 1):
                        c()

            if reps == 1:
                body(0)
            else:
                with tc.For_i(0, reps, 1, staggered_reset=True,
                              hint_engines=(mybir.EngineType.PE,)) as i:
                    body(i)

    nc.compile()
    return nc


def make_in_maps(X, W_qkv, W_out):
    """Host-side sharding: per-core input dict (bf16)."""
    import ml_dtypes
    BF = ml_dtypes.bfloat16

    X = np.asarray(X, dtype=np.float32)
    W_qkv = np.asarray(W_qkv, dtype=np.float32)
    W_out = np.asarray(W_out, dtype=np.float32)

    kp = np.arange(KT)[:, None]
    qf = np.arange(128)[None, :]
    masks = (qf >= kp).astype(BF)

    in_maps = []
    for c in range(NCORES):
        b, hg = divmod(c, NGROUP)
        rows = slice(hg * DLOC, (hg + 1) * DLOC)
        wq = W_qkv[0 * D:1 * D][rows].T * 0.125   # fold 1/sqrt(hd) into q
        wk = W_qkv[1 * D:2 * D][rows].T
        wv = W_qkv[2 * D:3 * D][rows].T
        in_maps.append({
            "xt": np.ascontiguousarray(X[b].T).astype(BF),
            "wqkv": np.ascontiguousarray(
                np.concatenate([wq, wk, wv], axis=1)).astype(BF),
            "wout": np.ascontiguousarray(W_out[:, rows].T).astype(BF),
            "masks": masks,
            "ones": np.ones((128, NKT, HPC, 1), dtype=BF),
        })
    return in_maps


def combine_outputs(results):
    """Sum the 4 tensor-parallel partials per batch -> [B, T, D]."""
    out = np.zeros((B, T, D), dtype=np.float32)
    for c, r in enumerate(results):
        out[c // NGROUP] += np.asarray(r["out"], dtype=np.float32)
    return out


_cached = {}


def kernel(X, W_qkv, W_out):
    from concourse.bass_utils import run_bass_kernel_spmd

    if "nc" not in _cached:
        _cached["nc"] = build_program(reps=1)
    nc = _cached["nc"]
    in_maps = make_in_maps(X, W_qkv, W_out)
    r = run_bass_kernel_spmd(nc, in_maps, core_ids=list(range(NCORES)))
    return combine_outputs(r.results)
